# revision 2
# baseline (speedup 1.0000x reference)
import numpy as np
import concourse.bass as bass
import concourse.bacc as bacc
import concourse.mybir as mybir
from concourse.bass_utils import run_bass_kernel_spmd
from concourse import tile

# DigitCapsules dynamic routing, data-parallel over batch on 8 cores.
# B=512, R=1152, C=10, O=16, I=8; per core Bl=64.
#
# Device layout: partitions p = parity*64 + b  (r = 2*rp + parity), so every
# per-(b,r) routing quantity is partition-local. u_hat for a chunk of r-pairs
# is generated by K=16 matmuls whose stationary operand is a host-built
# block-diagonal "pair canvas" [16, 128]: rows (parity, i), cols (parity, b).

NCORES = 8
B, R, C, O, I = 512, 1152, 10, 16, 8
Bl = B // NCORES          # 64 batch per core
CO = C * O                # 160
NP = R // 2               # 576 r-pairs
PAIRS_PER_CHUNK = 24      # 48 r per chunk; 8 psum banks x 3 pairs
NCHUNK = NP // PAIRS_PER_CHUNK  # 24
FCH = PAIRS_PER_CHUNK * CO      # 3840 free elems per chunk
EPS = 1e-8

_cache = {}


def _build_program():
    if "nc" in _cache:
        return _cache["nc"]
    nc = bacc.Bacc("TRN2", target_bir_lowering=False, debug=False)
    f32 = mybir.dt.float32
    canvas_d = nc.dram_tensor("canvas", [16, NP, 128], f32, kind="ExternalInput")
    wpair_d = nc.dram_tensor("wpair", [16, NP, CO], f32, kind="ExternalInput")
    out_d = nc.dram_tensor("v_out", [Bl, CO], f32, kind="ExternalOutput")

    AX = mybir.AxisListType
    ALU = mybir.AluOpType
    ACTF = mybir.ActivationFunctionType

    def ap(t, dims, offset=0):
        return bass.AP(t.tensor, offset, dims)

    with tile.TileContext(nc) as tc:
        with (
            tc.tile_pool(name="canv", bufs=3) as canv_pool,
            tc.tile_pool(name="wp", bufs=3) as wp_pool,
            tc.tile_pool(name="psum", bufs=8, space="PSUM") as psum_pool,
            tc.tile_pool(name="uch", bufs=2) as uch_pool,
            tc.tile_pool(name="tmp", bufs=2) as tmp_pool,
            tc.tile_pool(name="res", bufs=1) as res_pool,
        ):
            b_ij = res_pool.tile([128, NP * C], f32, tag="bij")       # logits, (rp, c)
            s_acc = res_pool.tile([128, CO], f32, tag="sacc")
            vtile = res_pool.tile([128, CO], f32, tag="vt")           # v broadcast both halves
            s_fold = res_pool.tile([64, CO], f32, tag="sfold")
            sq = res_pool.tile([64, C], f32, tag="sq")
            f1 = res_pool.tile([64, C], f32, tag="f1")
            f2 = res_pool.tile([64, C], f32, tag="f2")

            nc.vector.memset(b_ij[:], 0.0)
            bij_pitch = NP * C

            for t in range(3):
                nc.vector.memset(s_acc[:], 0.0)
                for k in range(NCHUNK):
                    cch = canv_pool.tile([16, PAIRS_PER_CHUNK * 128], f32, tag="c")
                    wch = wp_pool.tile([16, PAIRS_PER_CHUNK * CO], f32, tag="w")
                    nc.sync.dma_start(
                        cch[:],
                        canvas_d[:, k * PAIRS_PER_CHUNK:(k + 1) * PAIRS_PER_CHUNK, :],
                    )
                    nc.sync.dma_start(
                        wch[:],
                        wpair_d[:, k * PAIRS_PER_CHUNK:(k + 1) * PAIRS_PER_CHUNK, :],
                    )
                    uch = uch_pool.tile([128, FCH], f32, tag="u")
                    for j in range(8):  # 8 psum tiles, 3 pairs each
                        ps = psum_pool.tile([128, 3 * CO], f32, tag="ps")
                        for q in range(3):
                            rp = j * 3 + q
                            nc.tensor.matmul(
                                ps[:, q * CO:(q + 1) * CO],
                                cch[:, rp * 128:(rp + 1) * 128],
                                wch[:, rp * CO:(rp + 1) * CO],
                                start=True, stop=True,
                            )
                        eng = nc.vector if j % 2 == 0 else nc.scalar
                        if j % 2 == 0:
                            eng.tensor_copy(uch[:, j * 3 * CO:(j + 1) * 3 * CO], ps[:])
                        else:
                            eng.copy(uch[:, j * 3 * CO:(j + 1) * 3 * CO], ps[:])

                    # uch free dims: (rp 24, c 10, o 16); strides rp=160, c=16, o=1
                    upitch = FCH
                    if t > 0:
                        # a_{t-1}[p,(rp,c)] = sum_o u*v ; b_ij += a
                        tmp = tmp_pool.tile([128, FCH], f32, tag="m1")
                        nc.vector.tensor_tensor(
                            tmp[:],
                            uch[:],
                            ap(vtile, [[CO, 128], [0, PAIRS_PER_CHUNK], [1, CO]]),
                            op=ALU.mult,
                        )
                        ared = tmp_pool.tile([128, PAIRS_PER_CHUNK * C], f32, tag="ar")
                        nc.vector.tensor_reduce(
                            ared[:],
                            ap(tmp, [[FCH, 128], [CO, PAIRS_PER_CHUNK], [16, C], [1, O]]),
                            axis=AX.X, op=ALU.add,
                        )
                        bsl = b_ij[:, k * PAIRS_PER_CHUNK * C:(k + 1) * PAIRS_PER_CHUNK * C]
                        nc.vector.tensor_tensor(bsl, bsl, ared[:], op=ALU.add)
                        # softmax over c
                        cexp = tmp_pool.tile([128, PAIRS_PER_CHUNK * C], f32, tag="ce")
                        nc.scalar.activation(cexp[:], bsl, ACTF.Exp)
                        csum = tmp_pool.tile([128, PAIRS_PER_CHUNK], f32, tag="cs")
                        nc.vector.tensor_reduce(
                            csum[:],
                            ap(cexp, [[PAIRS_PER_CHUNK * C, 128], [C, PAIRS_PER_CHUNK], [1, C]]),
                            axis=AX.X, op=ALU.add,
                        )
                        crec = tmp_pool.tile([128, PAIRS_PER_CHUNK], f32, tag="cr")
                        nc.vector.reciprocal(crec[:], csum[:])
                        cij = tmp_pool.tile([128, PAIRS_PER_CHUNK * C], f32, tag="cij")
                        nc.vector.tensor_tensor(
                            cij[:],
                            cexp[:],
                            ap(crec, [[PAIRS_PER_CHUNK, 128], [1, PAIRS_PER_CHUNK], [0, C]]),
                            op=ALU.mult,
                        )
                        # s += sum_r cij * u
                        tmp2 = tmp_pool.tile([128, FCH], f32, tag="m2")
                        nc.vector.tensor_tensor(
                            tmp2[:],
                            uch[:],
                            ap(cij, [[PAIRS_PER_CHUNK * C, 128], [C, PAIRS_PER_CHUNK], [1, C], [0, O]]),
                            op=ALU.mult,
                        )
                        sred = tmp_pool.tile([128, CO], f32, tag="sr")
                        nc.vector.tensor_reduce(
                            sred[:],
                            ap(tmp2, [[FCH, 128], [16, C], [1, O], [CO, PAIRS_PER_CHUNK]]),
                            axis=AX.X, op=ALU.add,
                        )
                        nc.vector.tensor_tensor(s_acc[:], s_acc[:], sred[:], op=ALU.add)
                    else:
                        # uniform c = 1/10: s += sum_r u  (0.1 applied later)
                        sred = tmp_pool.tile([128, CO], f32, tag="sr")
                        nc.vector.tensor_reduce(
                            sred[:],
                            ap(uch, [[FCH, 128], [16, C], [1, O], [CO, PAIRS_PER_CHUNK]]),
                            axis=AX.X, op=ALU.add,
                        )
                        nc.vector.tensor_tensor(s_acc[:], s_acc[:], sred[:], op=ALU.add)

                # fold parity halves: s_fold = s_acc[0:64] + s_acc[64:128]
                upper = tmp_pool.tile([64, CO], f32, tag="up")
                nc.sync.dma_start(upper[:], s_acc[64:128, :])
                nc.vector.tensor_tensor(s_fold[:], s_acc[0:64, :], upper[:], op=ALU.add)
                if t == 0:
                    nc.vector.tensor_scalar_mul(s_fold[:], s_fold[:], 0.1)
                # squash: f = sq/(1+sq)/sqrt(sq+eps); v = f * s
                prod = tmp_pool.tile([64, CO], f32, tag="pr")
                nc.vector.tensor_tensor(prod[:], s_fold[:], s_fold[:], op=ALU.mult)
                nc.vector.tensor_reduce(
                    sq[:],
                    ap(prod, [[CO, 64], [16, C], [1, O]]),
                    axis=AX.X, op=ALU.add,
                )
                # f1 = 1/(1+sq); f2 = 1/sqrt(sq+eps); f = sq*f1*f2
                onep = tmp_pool.tile([64, C], f32, tag="q1")
                nc.vector.tensor_scalar_add(onep[:], sq[:], 1.0)
                nc.vector.reciprocal(f1[:], onep[:])
                rt = tmp_pool.tile([64, C], f32, tag="q2")
                nc.vector.tensor_scalar_add(rt[:], sq[:], EPS)
                nc.scalar.activation(rt[:], rt[:], ACTF.Sqrt)
                nc.vector.reciprocal(f2[:], rt[:])
                nc.vector.tensor_tensor(f1[:], f1[:], f2[:], op=ALU.mult)
                nc.vector.tensor_tensor(f1[:], f1[:], sq[:], op=ALU.mult)
                # v = s_fold * f1 (broadcast over o) -> vtile lower, then copy upper
                nc.vector.tensor_tensor(
                    vtile[0:64, :],
                    s_fold[:],
                    ap(f1, [[C, 64], [1, C], [0, O]]),
                    op=ALU.mult,
                )
                if t < 2:
                    nc.sync.dma_start(vtile[64:128, :], vtile[0:64, :])
                else:
                    nc.sync.dma_start(out_d[:], vtile[0:64, :])
    nc.compile()
    _cache["nc"] = nc
    return nc


def _host_prep(x, W):
    # x [B,R,I], W [1,R,C,O,I] -> canvases [16, NP, 128] per core, wpair [16, NP, CO]
    Wr = np.ascontiguousarray(
        W[0].reshape(R, CO, I).transpose(2, 0, 1), dtype=np.float32
    )  # [I, R, CO]
    wpair = np.zeros((16, NP, CO), np.float32)
    wpair[0:8] = Wr[:, 0::2, :]
    wpair[8:16] = Wr[:, 1::2, :]
    canvases = []
    for core in range(NCORES):
        xl = x[core * Bl:(core + 1) * Bl]          # [64, R, I]
        cv = np.zeros((16, NP, 128), np.float32)
        # rows (parity*8+i), cols (parity*64+b) = x[b, 2rp+parity, i]
        cv[0:8, :, 0:64] = xl[:, 0::2, :].transpose(2, 1, 0)
        cv[8:16, :, 64:128] = xl[:, 1::2, :].transpose(2, 1, 0)
        canvases.append(cv)
    return canvases, wpair


def kernel(x, W):
    x = np.asarray(x, dtype=np.float32)
    W = np.asarray(W, dtype=np.float32)
    nc = _build_program()
    canvases, wpair = _host_prep(x, W)
    in_maps = [{"canvas": canvases[i], "wpair": wpair} for i in range(NCORES)]
    res = run_bass_kernel_spmd(nc, in_maps, list(range(NCORES))).results
    out = np.concatenate([r["v_out"] for r in res], axis=0)  # [B, CO]
    return out.reshape(B, C, O)
